# revision 16
# baseline (speedup 1.0000x reference)
"""Causal multi-head attention on 8 Trainium2 NeuronCores (Bass/Tile).

Sharding: tensor-parallel over heads. Core i owns the 128 projected columns
(2 heads x 64) [128*i, 128*(i+1)): Wq/Wk/Wv split column-wise, Wo split
row-wise. Each core computes a full-[T, D] bf16 partial of the output
projection; the host sums the 8 partials in f32 and adds bo.

All matmul operands are bf16 (f32 PSUM accumulation). Per-core structure:
  - QKV projections produce Q^T/K^T/V^T [128, S] per batch (biases fused into
    the PSUM->SBUF evacuation on the vector engine; 1/sqrt(dh) folded into Wq).
  - V^T is transposed per 128-key chunk into V' [keys, 128] by the DMA xbar
    (16-bit transpose), with a ones column so the PV matmul also produces the
    softmax denominator.
  - Attention per 512-query tile over 128-key chunks, software-pipelined so
    the scalar engine's exp (the per-chunk bottleneck) overlaps PE work:
    per chunk issue scores S^T = K^T.T @ Q^T (two heads on disjoint 64-row
    PE groups), exp on ACT, causal-mask multiply on DVE (diagonal chunks),
    then the PREVIOUS chunk's PV matmuls plus a slice of background PE work
    (projections of batch b+1, output projection of batch b-1).
  - Denominator broadcast via a rank-1 PE matmul (ones weight) into PSUM,
    normalization multiply on DVE writes bf16 attnout^T.
  - Output projection attnout^T.T @ Wo_slice^T -> bf16 partial [T, D].
"""
from contextlib import ExitStack

import numpy as np

import concourse.bass as bass
import concourse.mybir as mybir
import concourse.tile as tile
from concourse import bacc
from concourse.bass import ts, ds
from concourse.bass_utils import run_bass_kernel_spmd

F32 = mybir.dt.float32
F32R = mybir.dt.float32r
BF16 = mybir.dt.bfloat16
AF = mybir.ActivationFunctionType
MULT = mybir.AluOpType.mult
ADD = mybir.AluOpType.add

B, S, D = 4, 2048, 1024
P = 128
DH = 64
KO = D // P        # 8 contraction chunks for projections
QTILE = 512
CH = 128
TTILE = 512
N_CORES = 8


def _build_nc(reps=1):
    T = B * S
    n_ttiles_b = S // TTILE
    n_qt = S // QTILE
    n_ch = S // CH
    diag_per_q = QTILE // CH

    nc = bacc.Bacc()
    xT = nc.declare_dram_parameter("xT", [D, T], BF16, isOutput=False)
    wqT = nc.declare_dram_parameter("wqT", [D, P], BF16, isOutput=False)
    wkT = nc.declare_dram_parameter("wkT", [D, P], BF16, isOutput=False)
    wvT = nc.declare_dram_parameter("wvT", [D, P], BF16, isOutput=False)
    woT = nc.declare_dram_parameter("woT", [P, D], BF16, isOutput=False)
    bqv = nc.declare_dram_parameter("bq", [P, 1], F32, isOutput=False)
    bkv = nc.declare_dram_parameter("bk", [P, 1], F32, isOutput=False)
    bvv = nc.declare_dram_parameter("bv", [P, 1], F32, isOutput=False)
    cmask = nc.declare_dram_parameter("cmask", [P, diag_per_q, QTILE], BF16,
                                      isOutput=False)
    vpad1 = nc.declare_dram_parameter("vpad1", [P, P], BF16, isOutput=False)
    vpad2 = nc.declare_dram_parameter("vpad2", [P, P], BF16, isOutput=False)
    wselp = nc.declare_dram_parameter("wsel", [P, P], F32R, isOutput=False)
    zqt = nc.declare_dram_parameter("zq", [P, QTILE], F32R, isOutput=False)
    out = nc.declare_dram_parameter("out", [T, D], BF16, isOutput=True)

    with tile.TileContext(nc) as tc, ExitStack() as ctx:
        const = ctx.enter_context(tc.tile_pool(name="const", bufs=1))
        bigp = ctx.enter_context(tc.tile_pool(name="big", bufs=2))
        xp = ctx.enter_context(tc.tile_pool(name="xp", bufs=2))
        ptp = ctx.enter_context(tc.tile_pool(name="pt", bufs=2))
        vp = ctx.enter_context(tc.tile_pool(name="vp", bufs=2))
        osp = ctx.enter_context(tc.tile_pool(name="os", bufs=2))
        ps = ctx.enter_context(tc.tile_pool(name="ps", bufs=2, space="PSUM"))

        wq_t = const.tile([P, KO, P], BF16, tag="wq")
        wk_t = const.tile([P, KO, P], BF16, tag="wk")
        wv_t = const.tile([P, KO, P], BF16, tag="wv")
        nc.sync.dma_start(out=wq_t, in_=wqT.rearrange("(ko ki) m -> ki ko m", ki=P))
        nc.sync.dma_start(out=wk_t, in_=wkT.rearrange("(ko ki) m -> ki ko m", ki=P))
        nc.sync.dma_start(out=wv_t, in_=wvT.rearrange("(ko ki) m -> ki ko m", ki=P))
        wo_t = const.tile([P, D], BF16, tag="wo")
        nc.sync.dma_start(out=wo_t, in_=woT[:, :])
        bq_t = const.tile([P, 1], F32, tag="bq")
        bk_t = const.tile([P, 1], F32, tag="bk")
        bv_t = const.tile([P, 1], F32, tag="bv")
        nc.sync.dma_start(out=bq_t, in_=bqv[:, :])
        nc.sync.dma_start(out=bk_t, in_=bkv[:, :])
        nc.sync.dma_start(out=bv_t, in_=bvv[:, :])
        cm_t = const.tile([P, diag_per_q, QTILE], BF16, tag="cm")
        nc.sync.dma_start(out=cm_t, in_=cmask[:, :, :])
        wsel_t = const.tile([P, P], F32R, tag="wsel")
        nc.sync.dma_start(out=wsel_t, in_=wselp[:, :])

        xT_r = xT.rearrange("(ko ki) t -> ki ko t", ki=P)

        rep_ctx = tc.For_i(0, reps, 1) if reps > 1 else None
        if rep_ctx is not None:
            ctx.enter_context(rep_ctx)

        pb = {}
        vb = {}
        ab_ = {}
        bg = []            # background closures: (pe_cost_ns, fn)

        def bg_drain(budget_ns):
            while bg and budget_ns > 0:
                cost, fn = bg.pop(0)
                fn()
                budget_ns -= cost

        def alloc_proj(b):
            qt_b = bigp.tile([P, S], BF16, tag="qt", name=f"qt{b}")
            kt_b = bigp.tile([P, S], BF16, tag="kt", name=f"kt{b}")
            vt_b = bigp.tile([P, S], BF16, tag="vt", name=f"vt{b}")
            pb[b] = (qt_b, kt_b, vt_b)

        def queue_proj(b, tt):
            """Queue projection of token tile tt of batch b as closures."""
            b0 = b * S
            xt = xp.tile([P, KO, TTILE], BF16, tag="xt", name=f"xt{b}_{tt}")

            def ld():
                nc.sync.dma_start(
                    out=xt, in_=xT_r[:, :, ds(b0 + tt * TTILE, TTILE)])
            bg.append((0, ld))
            for pi, (w_t, b_t, di) in enumerate((
                    (wq_t, bq_t, 0), (wk_t, bk_t, 1), (wv_t, bv_t, 2))):
                psm = ps.tile([P, TTILE], F32, tag="aux",
                              name=f"psm{b}_{tt}_{pi}")

                def mm(w_t=w_t, psm=psm, xt=xt, lo=0):
                    for ko in range(lo, lo + 4):
                        nc.tensor.matmul(psm, w_t[:, ko], xt[:, ko],
                                         start=(ko == 0), stop=(ko == KO - 1))

                def ev(psm=psm, b_t=b_t, dst_i=di, b=b, tt=tt):
                    dst = pb[b][dst_i]
                    nc.vector.tensor_scalar(out=dst[:, ts(tt, TTILE)], in0=psm,
                                            scalar1=b_t, scalar2=None, op0=ADD)
                bg.append((860, mm))
                bg.append((860, lambda mm=mm: mm(lo=4)))
                bg.append((270, ev))

        def queue_vbuild(b, tt):
            """Queue DMA-transposes of V^T token tile tt into V' chunks."""
            if tt == 0:
                v1 = vp.tile([P, n_ch, P], BF16, tag="v1", name=f"v1_{b}")
                v2 = vp.tile([P, n_ch, P], BF16, tag="v2", name=f"v2_{b}")
                vb[b] = (v1, v2)
                vp1_b = bass.AP(tensor=vpad1, offset=0,
                                ap=[[P, P], [0, n_ch], [1, P]])
                vp2_b = bass.AP(tensor=vpad2, offset=0,
                                ap=[[P, P], [0, n_ch], [1, P]])

                def pad():
                    nc.sync.dma_start(out=vb[b][0], in_=vp1_b)
                    nc.sync.dma_start(out=vb[b][1], in_=vp2_b)
                bg.append((0, pad))
            c0 = tt * (TTILE // CH)

            def tr(b=b, c0=c0):
                v1, v2 = vb[b]
                vt_b = pb[b][2]
                for c in range(c0, c0 + TTILE // CH):
                    nc.sync.dma_start(out=v1[:, c, 0:DH],
                                      in_=vt_b[0:DH, ts(c, CH)], transpose=True)
                    nc.sync.dma_start(out=v2[:, c, DH:P],
                                      in_=vt_b[DH:P, ts(c, CH)], transpose=True)
            bg.append((0, tr))

        def queue_outproj(b, j):
            """Queue output projection for the token range of qtile j."""
            ao_b = ab_[b]
            b0 = b * S
            for tt in range(j * (QTILE // P), (j + 1) * (QTILE // P)):
                for nn in range(D // QTILE):
                    def mmcp(b=b, b0=b0, tt=tt, nn=nn, ao_b=ao_b):
                        po = ps.tile([P, QTILE], F32, tag="aux",
                                     name=f"po{b}_{tt}_{nn}")
                        nc.tensor.matmul(po, ao_b[:, ts(tt, P)],
                                         wo_t[:, ts(nn, QTILE)],
                                         start=True, stop=True)
                        ot = osp.tile([P, QTILE], BF16, tag="ot", bufs=6,
                                      name=f"ot{b}_{tt}_{nn}")
                        nc.vector.tensor_copy(out=ot, in_=po)
                        nc.sync.dma_start(
                            out=out[ds(b0 + tt * P, P), ts(nn, QTILE)], in_=ot)
                    bg.append((215, mmcp))

        def attn_batch(b):
            qt_b, kt_b, _ = pb[b]
            v1, v2 = vb[b]
            ao_b = bigp.tile([P, S], BF16, tag="ao", name=f"ao{b}")
            ab_[b] = ao_b

            for j in range(n_qt):
                o1 = ps.tile([P, QTILE], F32, tag="o1", bufs=1,
                             name=f"o1_{b}_{j}")
                o2 = ps.tile([P, QTILE], F32, tag="o2", bufs=1,
                             name=f"o2_{b}_{j}")
                nch_j = (j + 1) * QTILE // CH
                pend = None        # (c, off, p12) awaiting PV issue

                def issue_pv(c, off, p12):
                    st, sp = (c == 0), (c == nch_j - 1)
                    nc.tensor.matmul(o1[:, off:], v1[:, c], p12[:, 0, off:],
                                     start=st, stop=sp)
                    nc.tensor.matmul(o2[:, off:], v2[:, c], p12[:, 1, off:],
                                     start=st, stop=sp)

                for c in range(nch_j):
                    di = c - j * diag_per_q
                    off = max(0, di) * CH
                    qs = ds(j * QTILE + off, QTILE - off)
                    s12 = ps.tile([P, 2, QTILE], F32, tag="s12",
                                  name=f"s12_{b}_{j}_{c}")
                    ksl = ds(c * CH, CH)
                    nc.tensor.matmul(s12[:, 0, off:], kt_b[0:DH, ksl],
                                     qt_b[0:DH, qs], start=True, stop=True)
                    nc.tensor.matmul(s12[:, 1, off:], kt_b[DH:P, ksl],
                                     qt_b[DH:P, qs], start=True, stop=True)
                    p12 = ptp.tile([P, 2, QTILE], BF16, tag="p12", bufs=4,
                                   name=f"p12_{b}_{j}_{c}")
                    nc.scalar.activation(out=p12[:, :, off:],
                                         in_=s12[:, :, off:], func=AF.Exp)
                    if di >= 0:
                        cmb = bass.AP(tensor=cm_t.tensor,
                                      offset=cm_t[:, di, off:].offset,
                                      ap=[cm_t.ap[0], [0, 2], [1, QTILE - off]])
                        nc.vector.tensor_tensor(out=p12[:, :, off:],
                                                in0=p12[:, :, off:],
                                                in1=cmb, op=MULT)
                    # previous chunk's PV + a slice of background work runs
                    # on PE while ACT computes this chunk's exp
                    if pend is not None:
                        issue_pv(*pend)
                        pend = None
                    bg_drain(800)
                    pend = (c, off, p12)
                # flush the last chunk's PV before normalization
                issue_pv(*pend)
                # normalization: reciprocal of denominator rows, PE rank-1
                # broadcast, DVE multiply into bf16 attnout^T
                stt = ptp.tile([P, QTILE], F32R, tag="st", name=f"st_{b}_{j}")
                if b == 0 and j < 2:
                    nc.sync.dma_start(out=stt, in_=zqt[:, :])
                with nc.allow_low_precision(reason="f32r holds full f32 bits"):
                    nc.vector.reciprocal(out=stt[DH:DH + 1], in_=o1[DH:DH + 1])
                    nc.vector.reciprocal(out=stt[32:33], in_=o2[32:33])
                dsb = ps.tile([P, QTILE], F32, tag="aux", name=f"dsb_{b}_{j}")
                nc.tensor.matmul(dsb, wsel_t, stt, start=True, stop=True)
                dsb_s = ptp.tile([P, QTILE], F32, tag="dsbs",
                                 name=f"dsbs_{b}_{j}")
                nc.vector.tensor_copy(out=dsb_s, in_=dsb)
                qsl = ds(j * QTILE, QTILE)
                nc.vector.tensor_tensor(out=ao_b[0:DH, qsl], in0=o1[0:DH],
                                        in1=dsb_s[0:DH], op=MULT)
                nc.vector.tensor_tensor(out=ao_b[DH:P, qsl], in0=o2[DH:P],
                                        in1=dsb_s[DH:P], op=MULT)
                queue_outproj(b, j)

        # batch 0 projections run upfront (nothing to hide them behind)
        alloc_proj(0)
        for tt in range(n_ttiles_b):
            queue_proj(0, tt)
            queue_vbuild(0, tt)
        bg_drain(1 << 30)
        for b in range(B):
            if b + 1 < B:
                alloc_proj(b + 1)
                for tt in range(n_ttiles_b):
                    queue_proj(b + 1, tt)
                    queue_vbuild(b + 1, tt)
            attn_batch(b)
            pb.pop(b, None)
        bg_drain(1 << 30)

    nc.compile()
    return nc


def _host_prepare(x, Wq, bq, Wk, bk, Wv, bv, Wo, bo):
    import ml_dtypes
    BF = ml_dtypes.bfloat16
    T = B * S
    scale = np.float32(1.0 / np.sqrt(np.float32(DH)))
    xT = np.ascontiguousarray(
        np.asarray(x, np.float32).reshape(T, D).T.astype(BF))

    k_idx = np.arange(CH)[:, None]
    q_idx = np.arange(QTILE)[None, :]
    cmaskv = np.stack(
        [(k_idx <= q_idx - off) for off in range(0, QTILE, CH)], axis=1
    ).astype(BF)

    vp1 = np.zeros((P, P), BF); vp1[:, DH] = 1.0
    vp2 = np.zeros((P, P), BF); vp2[:, 32] = 1.0
    wselv = np.zeros((P, P), np.float32)
    wselv[DH, 0:DH] = 1.0
    wselv[32, DH:P] = 1.0
    zqv = np.zeros((P, QTILE), np.float32)

    Wq = np.asarray(Wq, np.float32); Wk = np.asarray(Wk, np.float32)
    Wv = np.asarray(Wv, np.float32); Wo = np.asarray(Wo, np.float32)
    bq = np.asarray(bq, np.float32); bk = np.asarray(bk, np.float32)
    bv = np.asarray(bv, np.float32)

    in_maps = []
    for i in range(N_CORES):
        sl = slice(i * P, (i + 1) * P)
        in_maps.append({
            "xT": xT,
            "wqT": np.ascontiguousarray((Wq[sl].T * scale).astype(BF)),
            "wkT": np.ascontiguousarray(Wk[sl].T.astype(BF)),
            "wvT": np.ascontiguousarray(Wv[sl].T.astype(BF)),
            "woT": np.ascontiguousarray(Wo[:, sl].T.astype(BF)),
            "bq": (bq[sl] * scale).reshape(P, 1),
            "bk": bk[sl].reshape(P, 1).copy(),
            "bv": bv[sl].reshape(P, 1).copy(),
            "cmask": cmaskv,
            "vpad1": vp1,
            "vpad2": vp2,
            "wsel": wselv,
            "zq": zqv,
        })
    return in_maps


_NC_CACHE = {}


def kernel(x, Wq, bq, Wk, bk, Wv, bv, Wo, bo):
    if "nc" not in _NC_CACHE:
        _NC_CACHE["nc"] = _build_nc()
    nc = _NC_CACHE["nc"]
    in_maps = _host_prepare(x, Wq, bq, Wk, bk, Wv, bv, Wo, bo)
    res = run_bass_kernel_spmd(nc, in_maps, core_ids=list(range(N_CORES)))
    acc = res.results[0]["out"].astype(np.float32)
    for r in res.results[1:]:
        acc += r["out"].astype(np.float32)
    acc += np.asarray(bo, np.float32)
    return acc.reshape(B, S, D)


# revision 21
# speedup vs baseline: 1.2821x; 1.2821x over previous
"""Causal multi-head attention on 8 Trainium2 NeuronCores (Bass/Tile).

Sharding: tensor-parallel over heads. Core i owns the 128 projected columns
(2 heads x 64) [128*i, 128*(i+1)): Wq/Wk/Wv split column-wise, Wo split
row-wise. Each core computes a full-[T, D] bf16 partial of the output
projection; the host sums the 8 partials in f32 and adds bo.

All matmul operands are bf16 (f32 PSUM accumulation). Per-core structure:
  - QKV projections produce Q^T/K^T/V^T [128, S] per batch (biases fused into
    the PSUM->SBUF evacuation on the vector engine; 1/sqrt(dh) folded into Wq).
  - V^T is transposed per 128-key chunk into V' [keys, 128] by the DMA xbar
    (16-bit transpose), with a ones column so the PV matmul also produces the
    softmax denominator.
  - Attention per 512-query tile over 128-key chunks, software-pipelined one
    chunk deep so the scalar engine's exp (the per-chunk bottleneck) overlaps
    PE work: per chunk issue scores S^T = K^T.T @ Q^T (two heads on disjoint
    64-row PE groups), exp on ACT, causal-mask multiply on DVE (diagonal
    chunks), then the PREVIOUS chunk's PV matmuls and one slice of the
    previous batch's output projection.
  - Normalization: reciprocal of the denominator rows, partition-broadcast
    via a selection-matrix PE matmul, multiply into bf16 attnout^T.
  - Output projection attnout^T.T @ Wo_slice^T -> bf16 partial [T, D],
    interleaved into the next batch's attention chunk loop.
Projections of batch b+1 are issued per-qtile into the attention loop of
batch b (proven grain: psm shares the s12 PSUM slot rotation).
"""
from contextlib import ExitStack

import numpy as np

import concourse.bass as bass
import concourse.mybir as mybir
import concourse.tile as tile
from concourse import bacc
from concourse.bass import ts, ds
from concourse.bass_utils import run_bass_kernel_spmd

F32 = mybir.dt.float32
F32R = mybir.dt.float32r
BF16 = mybir.dt.bfloat16
AF = mybir.ActivationFunctionType
MULT = mybir.AluOpType.mult
ADD = mybir.AluOpType.add

B, S, D = 4, 2048, 1024
P = 128
DH = 64
KO = D // P        # 8 contraction chunks for projections
QTILE = 512
CH = 128
TTILE = 512
N_CORES = 8


def _build_nc(reps=1):
    T = B * S
    n_ttiles_b = S // TTILE
    n_qt = S // QTILE
    n_ch = S // CH
    diag_per_q = QTILE // CH

    nc = bacc.Bacc()
    xT = nc.declare_dram_parameter("xT", [D, T], BF16, isOutput=False)
    wqT = nc.declare_dram_parameter("wqT", [D, P], BF16, isOutput=False)
    wkT = nc.declare_dram_parameter("wkT", [D, P], BF16, isOutput=False)
    wvT = nc.declare_dram_parameter("wvT", [D, P], BF16, isOutput=False)
    woT = nc.declare_dram_parameter("woT", [P, D], BF16, isOutput=False)
    bqv = nc.declare_dram_parameter("bq", [P, 1], F32, isOutput=False)
    bkv = nc.declare_dram_parameter("bk", [P, 1], F32, isOutput=False)
    bvv = nc.declare_dram_parameter("bv", [P, 1], F32, isOutput=False)
    cmask = nc.declare_dram_parameter("cmask", [P, diag_per_q, QTILE], BF16,
                                      isOutput=False)
    vpad1 = nc.declare_dram_parameter("vpad1", [P, DH], BF16, isOutput=False)
    vpad2 = nc.declare_dram_parameter("vpad2", [P, DH], BF16, isOutput=False)
    wselp = nc.declare_dram_parameter("wsel", [P, P], F32R, isOutput=False)
    zqt = nc.declare_dram_parameter("zq", [P, QTILE], F32R, isOutput=False)
    out = nc.declare_dram_parameter("out", [T, D], BF16, isOutput=True)

    with tile.TileContext(nc) as tc, ExitStack() as ctx:
        const = ctx.enter_context(tc.tile_pool(name="const", bufs=1))
        bigp = ctx.enter_context(tc.tile_pool(name="big", bufs=2))
        xp = ctx.enter_context(tc.tile_pool(name="xp", bufs=2))
        ptp = ctx.enter_context(tc.tile_pool(name="pt", bufs=2))
        vp = ctx.enter_context(tc.tile_pool(name="vp", bufs=2))
        osp = ctx.enter_context(tc.tile_pool(name="os", bufs=2))
        ps = ctx.enter_context(tc.tile_pool(name="ps", bufs=2, space="PSUM"))

        wq_t = const.tile([P, KO, P], BF16, tag="wq")
        wk_t = const.tile([P, KO, P], BF16, tag="wk")
        wv_t = const.tile([P, KO, P], BF16, tag="wv")
        nc.sync.dma_start(out=wq_t, in_=wqT.rearrange("(ko ki) m -> ki ko m", ki=P))
        nc.sync.dma_start(out=wk_t, in_=wkT.rearrange("(ko ki) m -> ki ko m", ki=P))
        nc.sync.dma_start(out=wv_t, in_=wvT.rearrange("(ko ki) m -> ki ko m", ki=P))
        wo_t = const.tile([P, D], BF16, tag="wo")
        nc.sync.dma_start(out=wo_t, in_=woT[:, :])
        bq_t = const.tile([P, 1], F32, tag="bq")
        bk_t = const.tile([P, 1], F32, tag="bk")
        bv_t = const.tile([P, 1], F32, tag="bv")
        nc.sync.dma_start(out=bq_t, in_=bqv[:, :])
        nc.sync.dma_start(out=bk_t, in_=bkv[:, :])
        nc.sync.dma_start(out=bv_t, in_=bvv[:, :])
        cm_t = const.tile([P, diag_per_q, QTILE], BF16, tag="cm")
        nc.sync.dma_start(out=cm_t, in_=cmask[:, :, :])
        wsel_t = const.tile([P, P], F32R, tag="wsel")
        nc.sync.dma_start(out=wsel_t, in_=wselp[:, :])

        xT_r = xT.rearrange("(ko ki) t -> ki ko t", ki=P)

        rep_ctx = tc.For_i(0, reps, 1) if reps > 1 else None
        if rep_ctx is not None:
            ctx.enter_context(rep_ctx)

        pb = {}
        vb = {}
        ab_ = {}
        bg = []            # pending output-projection slices (closures)

        def alloc_proj(b):
            qt_b = bigp.tile([P, S], BF16, tag="qt", name=f"qt{b}")
            kt_b = bigp.tile([P, S], BF16, tag="kt", name=f"kt{b}")
            vt_b = bigp.tile([P, S], BF16, tag="vt", name=f"vt{b}")
            pb[b] = (qt_b, kt_b, vt_b)

        def proj_ttile(b, tt):
            qt_b, kt_b, vt_b = pb[b]
            b0 = b * S
            xt = xp.tile([P, KO, TTILE], BF16, tag="xt", name=f"xt{b}_{tt}")
            nc.sync.dma_start(out=xt, in_=xT_r[:, :, ds(b0 + tt * TTILE, TTILE)])
            for pi, (w_t, b_t, dst) in enumerate((
                    (wq_t, bq_t, qt_b), (wk_t, bk_t, kt_b), (wv_t, bv_t, vt_b))):
                psm = ps.tile([P, TTILE], F32, tag="s12",
                              name=f"psm{b}_{tt}_{pi}")
                for ko in range(KO):
                    nc.tensor.matmul(psm, w_t[:, ko], xt[:, ko],
                                     start=(ko == 0), stop=(ko == KO - 1))
                nc.vector.tensor_scalar(out=dst[:, ts(tt, TTILE)], in0=psm,
                                        scalar1=b_t, scalar2=None, op0=ADD)

        def valloc(b):
            v1 = vp.tile([P, n_ch, P], BF16, tag="v1", name=f"v1_{b}")
            v2 = vp.tile([P, n_ch, P], BF16, tag="v2", name=f"v2_{b}")
            vb[b] = (v1, v2)
            vp1_b = bass.AP(tensor=vpad1, offset=0,
                            ap=[[DH, P], [0, n_ch], [1, DH]])
            vp2_b = bass.AP(tensor=vpad2, offset=0,
                            ap=[[DH, P], [0, n_ch], [1, DH]])
            nc.sync.dma_start(out=v1[:, :, DH:P], in_=vp1_b)
            nc.sync.dma_start(out=v2[:, :, 0:DH], in_=vp2_b)

        def vbuild_tt(b, tt):
            v1, v2 = vb[b]
            vt_b = pb[b][2]
            for c in range(tt * (TTILE // CH), (tt + 1) * (TTILE // CH)):
                nc.sync.dma_start(out=v1[:, c, 0:DH],
                                  in_=vt_b[0:DH, ts(c, CH)], transpose=True)
                nc.sync.dma_start(out=v2[:, c, DH:P],
                                  in_=vt_b[DH:P, ts(c, CH)], transpose=True)

        def queue_outproj(b, j):
            ao_b = ab_[b]
            b0 = b * S
            for tt in range(j * (QTILE // P), (j + 1) * (QTILE // P)):
                for nn in range(D // QTILE):
                    def mmcp(b=b, b0=b0, tt=tt, nn=nn, ao_b=ao_b):
                        po = ps.tile([P, QTILE], F32, tag="aux",
                                     name=f"po{b}_{tt}_{nn}")
                        nc.tensor.matmul(po, ao_b[:, ts(tt, P)],
                                         wo_t[:, ts(nn, QTILE)],
                                         start=True, stop=True)
                        ot = osp.tile([P, QTILE], BF16, tag="ot", bufs=6,
                                      name=f"ot{b}_{tt}_{nn}")
                        nc.vector.tensor_copy(out=ot, in_=po)
                        nc.sync.dma_start(
                            out=out[ds(b0 + tt * P, P), ts(nn, QTILE)], in_=ot)
                    bg.append(mmcp)

        def attn_batch(b):
            qt_b, kt_b, _ = pb[b]
            v1, v2 = vb[b]
            ao_b = bigp.tile([P, S], BF16, tag="ao", name=f"ao{b}")
            ab_[b] = ao_b
            if b + 1 < B:
                valloc(b + 1)

            for j in range(n_qt):
                # projections + V' build of batch b+1, one token tile per qtile
                if b + 1 < B and j < n_ttiles_b:
                    proj_ttile(b + 1, j)
                    vbuild_tt(b + 1, j)
                o1 = ps.tile([P, QTILE], F32, tag="o1", bufs=1,
                             name=f"o1_{b}_{j}")
                o2 = ps.tile([P, QTILE], F32, tag="o2", bufs=1,
                             name=f"o2_{b}_{j}")
                nch_j = (j + 1) * QTILE // CH
                pend = None        # (c, off, p12) awaiting PV issue

                def issue_pv(c, off, p12):
                    st, sp = (c == 0), (c == nch_j - 1)
                    nc.tensor.matmul(o1[:, off:], v1[:, c], p12[:, 0, off:],
                                     start=st, stop=sp)
                    nc.tensor.matmul(o2[:, off:], v2[:, c], p12[:, 1, off:],
                                     start=st, stop=sp)

                for c in range(nch_j):
                    di = c - j * diag_per_q
                    off = max(0, di) * CH
                    qs = ds(j * QTILE + off, QTILE - off)
                    s12 = ps.tile([P, 2, QTILE], F32, tag="s12",
                                  name=f"s12_{b}_{j}_{c}")
                    ksl = ds(c * CH, CH)
                    nc.tensor.matmul(s12[:, 0, off:], kt_b[0:DH, ksl],
                                     qt_b[0:DH, qs], start=True, stop=True)
                    nc.tensor.matmul(s12[:, 1, off:], kt_b[DH:P, ksl],
                                     qt_b[DH:P, qs], start=True, stop=True)
                    p12 = ptp.tile([P, 2, QTILE], BF16, tag="p12", bufs=4,
                                   name=f"p12_{b}_{j}_{c}")
                    nc.scalar.activation(out=p12[:, :, off:],
                                         in_=s12[:, :, off:], func=AF.Exp)
                    if di >= 0:
                        cmb = bass.AP(tensor=cm_t.tensor,
                                      offset=cm_t[:, di, off:].offset,
                                      ap=[cm_t.ap[0], [0, 2], [1, QTILE - off]])
                        nc.vector.tensor_tensor(out=p12[:, :, off:],
                                                in0=p12[:, :, off:],
                                                in1=cmb, op=MULT)
                    # previous chunk's PV + one outproj slice run on PE while
                    # ACT computes this chunk's exp
                    if pend is not None:
                        issue_pv(*pend)
                        pend = None
                    if bg:
                        bg.pop(0)()
                    pend = (c, off, p12)
                issue_pv(*pend)
                # normalization: reciprocal of denominator rows, PE broadcast
                # via selection matrix, DVE multiply into bf16 attnout^T
                stt = ptp.tile([P, QTILE], F32R, tag="st", name=f"st_{b}_{j}")
                nc.vector.memset(stt[:, :].bitcast(F32), 0)
                with nc.allow_low_precision(reason="f32r holds full f32 bits"):
                    nc.vector.reciprocal(out=stt[DH:DH + 1], in_=o1[DH:DH + 1])
                    nc.vector.reciprocal(out=stt[32:33], in_=o2[32:33])
                dsb = ps.tile([P, QTILE], F32, tag="aux", name=f"dsb_{b}_{j}")
                nc.tensor.matmul(dsb, wsel_t, stt, start=True, stop=True)
                dsb_s = ptp.tile([P, QTILE], F32, tag="dsbs",
                                 name=f"dsbs_{b}_{j}")
                nc.vector.tensor_copy(out=dsb_s, in_=dsb)
                qsl = ds(j * QTILE, QTILE)
                nc.vector.tensor_tensor(out=ao_b[0:DH, qsl], in0=o1[0:DH],
                                        in1=dsb_s[0:DH], op=MULT)
                nc.vector.tensor_tensor(out=ao_b[DH:P, qsl], in0=o2[DH:P],
                                        in1=dsb_s[DH:P], op=MULT)
                queue_outproj(b, j)

        # batch 0 projections run upfront (nothing to hide them behind)
        alloc_proj(0)
        valloc(0)
        for tt in range(n_ttiles_b):
            proj_ttile(0, tt)
            vbuild_tt(0, tt)
        for b in range(B):
            if b + 1 < B:
                alloc_proj(b + 1)
            attn_batch(b)
            pb.pop(b, None)
        while bg:
            bg.pop(0)()

    nc.compile()
    return nc


def _host_prepare(x, Wq, bq, Wk, bk, Wv, bv, Wo, bo):
    import ml_dtypes
    BF = ml_dtypes.bfloat16
    T = B * S
    scale = np.float32(1.0 / np.sqrt(np.float32(DH)))
    xT = np.ascontiguousarray(
        np.asarray(x, np.float32).reshape(T, D).T.astype(BF))

    k_idx = np.arange(CH)[:, None]
    q_idx = np.arange(QTILE)[None, :]
    cmaskv = np.stack(
        [(k_idx <= q_idx - off) for off in range(0, QTILE, CH)], axis=1
    ).astype(BF)

    vp1 = np.zeros((P, DH), BF); vp1[:, 0] = 1.0   # -> v1 cols 64..127
    vp2 = np.zeros((P, DH), BF); vp2[:, 32] = 1.0  # -> v2 cols 0..63
    wselv = np.zeros((P, P), np.float32)
    wselv[DH, 0:DH] = 1.0
    wselv[32, DH:P] = 1.0
    zqv = np.zeros((P, QTILE), np.float32)

    Wq = np.asarray(Wq, np.float32); Wk = np.asarray(Wk, np.float32)
    Wv = np.asarray(Wv, np.float32); Wo = np.asarray(Wo, np.float32)
    bq = np.asarray(bq, np.float32); bk = np.asarray(bk, np.float32)
    bv = np.asarray(bv, np.float32)

    in_maps = []
    for i in range(N_CORES):
        sl = slice(i * P, (i + 1) * P)
        in_maps.append({
            "xT": xT,
            "wqT": np.ascontiguousarray((Wq[sl].T * scale).astype(BF)),
            "wkT": np.ascontiguousarray(Wk[sl].T.astype(BF)),
            "wvT": np.ascontiguousarray(Wv[sl].T.astype(BF)),
            "woT": np.ascontiguousarray(Wo[:, sl].T.astype(BF)),
            "bq": (bq[sl] * scale).reshape(P, 1),
            "bk": bk[sl].reshape(P, 1).copy(),
            "bv": bv[sl].reshape(P, 1).copy(),
            "cmask": cmaskv,
            "vpad1": vp1,
            "vpad2": vp2,
            "wsel": wselv,
            "zq": zqv,
        })
    return in_maps


_NC_CACHE = {}


def kernel(x, Wq, bq, Wk, bk, Wv, bv, Wo, bo):
    if "nc" not in _NC_CACHE:
        _NC_CACHE["nc"] = _build_nc()
    nc = _NC_CACHE["nc"]
    in_maps = _host_prepare(x, Wq, bq, Wk, bk, Wv, bv, Wo, bo)
    res = run_bass_kernel_spmd(nc, in_maps, core_ids=list(range(N_CORES)))
    acc = res.results[0]["out"].astype(np.float32)
    for r in res.results[1:]:
        acc += r["out"].astype(np.float32)
    acc += np.asarray(bo, np.float32)
    return acc.reshape(B, S, D)
